# revision 33
# baseline (speedup 1.0000x reference)
"""Trainium2 Bass kernel for the Nawrot downsampler-upsampler module.

Per-core (data-parallel over batch, 1 example per NeuronCore):
  1. load x, round to fp16, XBAR-DMA-transpose -> fp16 MLP (relu(x@W1+b1)),
     logits via ones-matmul partition reduce (fast path, ~4e-4 logit err)
  2. boundary bits from logits + logistic noise; borderline tokens
     (|z| < tau) recomputed exactly in fp32 via indirect gather + wide
     matmuls, corrected bits scattered back (kills fp16 flip risk)
  3. prefix-max scans give per-token segment end e(l), prev boundary s(l),
     reciprocal count r(l)
  4. banded matmul: up[l] = sum_j M[l,j] x[j] with M[l,j] = r(l) for
     s(l) < j <= e(l). Segments are short (max look-back ~12 tokens), so
     per output tile tl only j-tiles {tl-1, tl} contribute.
  5. null tokens (before first boundary, only possible in tile 0) get
     null_group via a masked blend.
"""
import sys

sys.path.insert(0, "/opt/trn_rl_repo")

import numpy as np
from contextlib import ExitStack

import concourse.bass as bass
import concourse.bacc as bacc
import concourse.tile as tile
from concourse import mybir
from concourse.masks import make_identity

F32 = mybir.dt.float32
F16 = mybir.dt.float16
I32 = mybir.dt.int32
OP = mybir.AluOpType
ACT = mybir.ActivationFunctionType

B = 8
L_FULL = 2048
D_FULL = 1024
N_CORES = 8
K_REF = 32          # refinement slots (8 per 512-token chunk)
TAU = 4e-3          # borderline threshold on |logits + logistic|
BIG = 1.0e6


def build(L=L_FULL, D=D_FULL):
    P = 128
    NLT = L // P          # l-tiles (16)
    ND = D // P           # d-tiles (8)
    CPT = L // P          # scan columns per partition (l = p*CPT + c) (16)
    LCH = 512             # l-chunk for MLP matmuls
    NLC = L // LCH        # 4

    nc = bacc.Bacc("TRN2", target_bir_lowering=False, debug=False, num_devices=N_CORES)

    x_d = nc.dram_tensor("x", [L, D], F32, kind="ExternalInput").ap()
    noise_d = nc.dram_tensor("noise", [L], F32, kind="ExternalInput").ap()
    w1_d = nc.dram_tensor("W1", [D, D], F32, kind="ExternalInput").ap()
    b1_d = nc.dram_tensor("b1", [D], F32, kind="ExternalInput").ap()
    w2_d = nc.dram_tensor("W2", [D], F32, kind="ExternalInput").ap()
    b2_d = nc.dram_tensor("b2", [1], F32, kind="ExternalInput").ap()
    null_d = nc.dram_tensor("null_group", [1, 1, D], F32, kind="ExternalInput").ap()
    up_d = nc.dram_tensor("up", [L, D], F32, kind="ExternalOutput").ap()

    with tile.TileContext(nc) as tc, ExitStack() as ctx:
        const = ctx.enter_context(tc.tile_pool(name="const", bufs=1))
        dram = ctx.enter_context(tc.tile_pool(name="dram", bufs=1, space="DRAM"))

        # ---------------- DRAM scratch ----------------
        lg_scr = dram.tile([L, 1], F32)      # logits row bounce
        rank_scr = dram.tile([32, 1], F32)   # per-chunk rank relayout bounce
        rank2_scr = dram.tile([32, 1], F32)
        lgst_scr = dram.tile([L, 1], F32)    # logistic noise per token
        hard_scr = dram.tile([L, 1], F32)    # boundary bits (refine-corrected)
        slots_scr = dram.tile([K_REF, 1], I32)  # borderline token ids
        e_scr = dram.tile([L, 1], F32)       # e(l) = last boundary <= l
        s_scr = dram.tile([L, 1], F32)       # s(l) = boundary before e(l)
        r_scr = dram.tile([L, 1], F32)       # 1/(e-s+1e-9)

        # ---------------- constants ----------------
        const_dmas = []
        # W1 loaded per output-block, issued AFTER chunk-0 x loads (see
        # load_w1 below) so the first transposes aren't queued behind 4MB
        w1_sb = const.tile([P, ND, D], F32)
        w1h_sb = const.tile([P, ND, D], F16)
        w1_r = w1_d.rearrange("(i p) n -> p i n", p=P)

        def load_w1():
            for o in range(ND):
                osl = slice(o * P, (o + 1) * P)
                nc.sync.dma_start(out=w1_sb[:, :, osl], in_=w1_r[:, :, osl])
                nc.vector.tensor_copy(out=w1h_sb[:, :, osl], in_=w1_sb[:, :, osl])

        b1_sb = const.tile([P, ND], F32)
        const_dmas.append(nc.sync.dma_start(out=b1_sb[:], in_=b1_d.rearrange("(o p) -> p o", p=P)))
        w2_sb = const.tile([P, ND], F32)
        const_dmas.append(nc.sync.dma_start(out=w2_sb[:], in_=w2_d.rearrange("(o p) -> p o", p=P)))
        b2_sb = const.tile([1, 1], F32)
        const_dmas.append(nc.sync.dma_start(out=b2_sb[:], in_=b2_d.rearrange("(a b) -> a b", a=1)))
        null_sb = const.tile([1, D], F32)
        const_dmas.append(nc.sync.dma_start(out=null_sb[:], in_=null_d[0, 0, :].rearrange("(a d) -> a d", a=1)))
        b1row = const.tile([1, D], F32)
        const_dmas.append(nc.sync.dma_start(out=b1row[:], in_=b1_d.rearrange("(a d) -> a d", a=1)))
        w2row = const.tile([1, D], F32)
        const_dmas.append(nc.sync.dma_start(out=w2row[:], in_=w2_d.rearrange("(a d) -> a d", a=1)))

        ident = const.tile([P, P], F32)
        make_identity(nc, ident[:])
        identH = const.tile([P, P], F16)
        nc.vector.tensor_copy(out=identH[:], in_=ident[:])

        pio = const.tile([P, 1], F32)
        nc.gpsimd.iota(pio[:], pattern=[[0, 1]], base=0, channel_multiplier=1,
                       allow_small_or_imprecise_dtypes=True)
        # jv16[p, t] = p + 128*t  (token id of partition-row p in l-tile t)
        jv16 = const.tile([P, NLT], F32)
        nc.gpsimd.iota(jv16[:], pattern=[[P, NLT]], base=0, channel_multiplier=1,
                       allow_small_or_imprecise_dtypes=True)
        # iot16[p, c] = 16p + c  (token id in scan layout)
        iot16 = const.tile([P, CPT], F32)
        nc.gpsimd.iota(iot16[:], pattern=[[1, CPT]], base=0, channel_multiplier=CPT,
                       allow_small_or_imprecise_dtypes=True)
        iotp1 = const.tile([P, CPT], F32)   # l + 1
        nc.gpsimd.iota(iotp1[:], pattern=[[1, CPT]], base=1, channel_multiplier=CPT,
                       allow_small_or_imprecise_dtypes=True)

        ones_row1 = const.tile([1, P], F32)
        nc.vector.memset(ones_row1[:], 1.0)
        ones_row_h = const.tile([1, P], F16)
        nc.vector.memset(ones_row_h[:], 1.0)
        ones_col = const.tile([P, 1], F32)
        nc.vector.memset(ones_col[:], 1.0)
        ones_1x1 = const.tile([1, 1], F32)
        nc.vector.memset(ones_1x1[:], 1.0)
        ones_1x1h = const.tile([1, 1], F16)
        nc.vector.memset(ones_1x1h[:], 1.0)
        zero_col = const.tile([P, 1], F32)
        nc.vector.memset(zero_col[:], 0.0)
        zeros_cpt = const.tile([P, CPT], F32)
        nc.vector.memset(zeros_cpt[:], 0.0)
        zrow128 = const.tile([1, P], F32)
        nc.vector.memset(zrow128[:], 0.0)
        zslots = const.tile([K_REF, 1], I32)
        nc.vector.memset(zslots[:], 0.0)
        const_dmas.append(nc.sync.dma_start(out=slots_scr[:], in_=zslots[:]))
        zeros_kd = const.tile([K_REF, D], F32)
        nc.vector.memset(zeros_kd[:], 0.0)

        # z-pipeline tiles in scan layout [p, c] with l = 16p + c; chunk lc
        # of 512 tokens maps exactly to partitions [32*lc, 32*(lc+1)), so the
        # pipeline runs per-chunk inside phase A shadows
        lg16 = const.tile([P, CPT], F32)
        nz16 = const.tile([P, CPT], F32)
        lnu = const.tile([P, CPT], F32)
        om = const.tile([P, CPT], F32)
        ln1m = const.tile([P, CPT], F32)
        lgst = const.tile([P, CPT], F32)
        zt = const.tile([P, CPT], F32)
        hard = const.tile([P, CPT], F32)
        mlo = const.tile([P, CPT], F32)
        mhi = const.tile([P, CPT], F32)
        mm = const.tile([P, CPT], F32)
        lr = const.tile([P, CPT], F32)
        off_col = const.tile([P, 1], F32)
        sel_eq = const.tile([P, CPT], F32)
        sel_t = const.tile([P, CPT], F32)
        sel_ts = const.tile([P, CPT], F32)
        nc.vector.memset(sel_ts[:], 0.0)
        sel_pr = const.tile([P, CPT], F32)
        sel_sl = const.tile([P, 1], F32)
        nc.vector.memset(sel_sl[:], BIG)
        sel_sli = const.tile([P, 1], I32)
        sel_tk = const.tile([P, 1], I32)
        # refine gather targets (persistent across phase A -> B)
        slots_sb = const.tile([K_REF, 1], I32)
        xg = const.tile([K_REF, D], F32)
        lgst_g = const.tile([K_REF, 1], F32)

        # x rounded to fp16, resident for XBAR transposes + banded matmul
        x_h = const.tile([P, NLT, D], F16)

        # shared bounds-check registers for indirect DMAs
        bc_rows = nc.gpsimd.to_reg(L - 1)
        bc_slot = nc.gpsimd.to_reg(K_REF - 1)

        # Collapse constant-load DMA lanes (keep sync-wait slots per
        # instruction under the limit).
        from concourse.tile_rust import add_dep_helper as _adh
        for g in range(0, len(const_dmas), 4):
            spn = nc.sync.nop()
            for d in const_dmas[g:g + 4]:
                _adh(spn.ins, d.ins, sync=True, reason="const-lane coalesce")

        # broadcast helpers for the refine epilogue: b1/W2 replicated to
        # K_REF partitions, plus the null row replicated to 128
        with tc.tile_pool(name="pnull", bufs=1, space="PSUM") as pnull:
            null_bc = const.tile([P, D], F32)
            b1_bc = const.tile([K_REF, D], F32)
            w2_bc = const.tile([K_REF, D], F32)
            b2_bc = const.tile([K_REF, 1], F32)
            psb2 = pnull.tile([P, 512], F32, tag="b2")
            nc.tensor.matmul(psb2[0:K_REF, 0:1], lhsT=ones_row1[0:1, 0:K_REF],
                             rhs=b2_sb[:], start=True, stop=True)
            nc.vector.tensor_copy(out=b2_bc[:], in_=psb2[0:K_REF, 0:1])
            for h in range(2):
                hsl = slice(h * 512, (h + 1) * 512)
                psn = pnull.tile([P, 512], F32, tag="n")
                nc.tensor.matmul(psn[:], lhsT=ones_row1[:], rhs=null_sb[0:1, hsl],
                                 start=True, stop=True)
                nc.vector.tensor_copy(out=null_bc[:, hsl], in_=psn[:])
                psb = pnull.tile([P, 512], F32, tag="b")
                nc.tensor.matmul(psb[0:K_REF, :], lhsT=ones_row1[0:1, 0:K_REF],
                                 rhs=b1row[0:1, hsl], start=True, stop=True)
                nc.vector.tensor_copy(out=b1_bc[:, hsl], in_=psb[0:K_REF, :])
                psw = pnull.tile([P, 512], F32, tag="w")
                nc.tensor.matmul(psw[0:K_REF, :], lhsT=ones_row1[0:1, 0:K_REF],
                                 rhs=w2row[0:1, hsl], start=True, stop=True)
                nc.vector.tensor_copy(out=w2_bc[:, hsl], in_=psw[0:K_REF, :])

        # ------- phase A: per 512-token chunk: load, round, transpose, MLP -------
        with tc.tile_pool(name="xp", bufs=3) as xpool, \
             tc.tile_pool(name="xtp", bufs=2) as xtp, \
             tc.tile_pool(name="stage", bufs=3) as stage, \
             tc.tile_pool(name="logp", bufs=2) as logp, \
             tc.tile_pool(name="psT", bufs=2, space="PSUM") as psT, \
             tc.tile_pool(name="psM", bufs=2, space="PSUM") as psM:
            for lc in range(NLC):
                lsl = slice(lc * LCH, (lc + 1) * LCH)
                xT_ch = xtp.tile([P, ND, LCH], F16, tag="xT")

                for ii in range(LCH // P):
                    i = lc * (LCH // P) + ii
                    x_t = xpool.tile([P, D], F32, tag="x")
                    nc.sync.dma_start(out=x_t[:], in_=x_d[i * P:(i + 1) * P, :])
                    if lc == 0 and ii == 3:
                        load_w1()
                    # round to fp16 (scalar engine)
                    nc.scalar.activation(out=x_h[:, i, :], in_=x_t[:],
                                         func=ACT.Identity, bias=zero_col[:], scale=1.0)
                    # PE transposes (fp32 in/out, rounded to fp16 at the copy)
                    for jg in range((ND + 3) // 4):
                        n_in_g = min(4, ND - jg * 4)
                        ps_t = psT.tile([P, 512], F32, tag="tr")
                        for jj in range(n_in_g):
                            j = jg * 4 + jj
                            nc.tensor.transpose(
                                out=ps_t[:, jj * P:(jj + 1) * P],
                                in_=x_t[:, j * P:(j + 1) * P],
                                identity=ident[:],
                            )
                        nc.vector.tensor_copy(
                            out=xT_ch[:, jg * 4:jg * 4 + n_in_g, ii * P:(ii + 1) * P],
                            in_=ps_t[:, :n_in_g * P].rearrange("p (j q) -> p j q", q=P),
                        )

                # MLP for this l-chunk (fp16)
                logacc = logp.tile([P, LCH], F32, tag="logacc")
                for o in range(ND):
                    psm = psM.tile([P, LCH], F32, tag="mlp")
                    for i_ in range(ND):
                        nc.tensor.matmul(
                            psm[:],
                            lhsT=w1h_sb[:, i_, o * P:(o + 1) * P],
                            rhs=xT_ch[:, i_, :],
                            start=(i_ == 0), stop=(i_ == ND - 1),
                        )
                    hT = stage.tile([P, LCH], F32, tag="hT")
                    nc.scalar.activation(
                        out=hT[:], in_=psm[:], func=ACT.Relu,
                        bias=b1_sb[:, o:o + 1], scale=1.0,
                    )
                    if o == 0:
                        nc.vector.tensor_scalar(
                            out=logacc[:], in0=hT[:],
                            scalar1=w2_sb[:, o:o + 1], scalar2=None, op0=OP.mult,
                        )
                    else:
                        nc.vector.scalar_tensor_tensor(
                            out=logacc[:], in0=hT[:], scalar=w2_sb[:, o:o + 1],
                            in1=logacc[:], op0=OP.mult, op1=OP.add,
                        )

                pslg = psM.tile([1, LCH], F32, tag="lgr")
                nc.tensor.matmul(pslg[:], lhsT=ones_col[:], rhs=logacc[:], start=True, stop=True)
                lg_ch = stage.tile([1, LCH], F32, tag="lgch")
                nc.scalar.activation(
                    out=lg_ch[:], in_=pslg[:], func=ACT.Identity,
                    bias=b2_sb[:, 0:1], scale=1.0,
                )
                nc.sync.dma_start(
                    out=lg_scr[lsl, 0].rearrange("(a l) -> a l", a=1), in_=lg_ch[:]
                )

                # per-chunk z pipeline + borderline selection (scan layout
                # partitions [32*lc, 32*(lc+1)) == tokens [512*lc, 512*(lc+1)))
                sp = slice(32 * lc, 32 * (lc + 1))
                nc.sync.dma_start(out=lg16[sp, :],
                                  in_=lg_scr[lsl, 0].rearrange("(p c) -> p c", c=CPT))
                nc.sync.dma_start(out=nz16[sp, :],
                                  in_=noise_d[lsl].rearrange("(p c) -> p c", c=CPT))
                nc.scalar.activation(out=lnu[sp, :], in_=nz16[sp, :], func=ACT.Ln)
                nc.vector.tensor_scalar(out=om[sp, :], in0=nz16[sp, :], scalar1=1.0,
                                        scalar2=-1.0, op0=OP.subtract, op1=OP.mult)
                nc.scalar.activation(out=ln1m[sp, :], in_=om[sp, :], func=ACT.Ln)
                nc.vector.tensor_tensor(out=lgst[sp, :], in0=lnu[sp, :], in1=ln1m[sp, :],
                                        op=OP.subtract)
                nc.sync.dma_start(out=lgst_scr[lsl, 0].rearrange("(p c) -> p c", c=CPT),
                                  in_=lgst[sp, :])
                nc.vector.tensor_tensor(out=zt[sp, :], in0=lgst[sp, :], in1=lg16[sp, :], op=OP.add)
                nc.vector.tensor_scalar(out=hard[sp, :], in0=zt[sp, :], scalar1=0.0,
                                        scalar2=None, op0=OP.is_gt)
                nc.sync.dma_start(out=hard_scr[lsl, 0].rearrange("(p c) -> p c", c=CPT),
                                  in_=hard[sp, :])
                nc.vector.tensor_scalar(out=mlo[sp, :], in0=zt[sp, :], scalar1=TAU,
                                        scalar2=None, op0=OP.is_lt)
                nc.vector.tensor_scalar(out=mhi[sp, :], in0=zt[sp, :], scalar1=-TAU,
                                        scalar2=None, op0=OP.is_gt)
                nc.vector.tensor_tensor(out=mm[sp, :], in0=mlo[sp, :], in1=mhi[sp, :], op=OP.mult)
                nc.vector.tensor_tensor_scan(
                    out=lr[sp, :], data0=mm[sp, :], data1=zeros_cpt[sp, :],
                    initial=0.0, op0=OP.add, op1=OP.add,
                )

                # within-chunk exclusive rank offsets over the 32 partitions,
                # relayout via tiny DRAM bounces (no PE involved)
                nc.sync.dma_start(out=rank_scr[:],
                                  in_=lr[sp, CPT - 1:CPT])
                rowT = logp.tile([1, 32], F32, tag="selrow")
                nc.sync.dma_start(out=rowT[:],
                                  in_=rank_scr[:, 0].rearrange("(a k) -> a k", a=1))
                scn = logp.tile([1, 32], F32, tag="selscn")
                nc.vector.tensor_tensor_scan(
                    out=scn[:], data0=rowT[:], data1=zrow128[0:1, 0:32],
                    initial=0.0, op0=OP.add, op1=OP.add,
                )
                exc = logp.tile([1, 32], F32, tag="selexc")
                nc.vector.memset(exc[0:1, 0:1], 0.0)
                nc.vector.tensor_copy(out=exc[0:1, 1:32], in_=scn[0:1, 0:31])
                nc.sync.dma_start(out=rank2_scr[:, 0].rearrange("(a k) -> a k", a=1),
                                  in_=exc[:])
                nc.sync.dma_start(out=off_col[sp, :], in_=rank2_scr[:])

                # scatter up to 8 borderline token ids into this chunk's slots
                for j in (1, 2, 3):
                    nc.vector.tensor_scalar(out=sel_eq[sp, :], in0=lr[sp, :],
                                            scalar1=float(j), scalar2=None, op0=OP.is_equal)
                    nc.vector.tensor_tensor(out=sel_eq[sp, :], in0=sel_eq[sp, :],
                                            in1=mm[sp, :], op=OP.mult)
                    nc.vector.tensor_tensor(out=sel_t[sp, :], in0=iot16[sp, :],
                                            in1=sel_eq[sp, :], op=OP.mult)
                    nc.vector.tensor_tensor_scan(
                        out=sel_ts[sp, :], data0=sel_t[sp, :], data1=zeros_cpt[sp, :],
                        initial=0.0, op0=OP.add, op1=OP.add,
                    )
                    nc.vector.tensor_tensor_scan(
                        out=sel_pr[sp, :], data0=sel_eq[sp, :], data1=zeros_cpt[sp, :],
                        initial=0.0, op0=OP.add, op1=OP.add,
                    )
                    # in-chunk slot = off + j - 1; valid iff < 8 and present
                    nc.vector.tensor_scalar(out=sel_sl[sp, :], in0=off_col[sp, :],
                                            scalar1=float(j - 1), scalar2=None, op0=OP.add)
                    vld = logp.tile([P, 1], F32, tag="selvld")
                    nc.vector.tensor_scalar(out=vld[sp, :], in0=sel_sl[sp, :],
                                            scalar1=float(K_REF // NLC) - 0.5,
                                            scalar2=None, op0=OP.is_lt)
                    nc.vector.tensor_tensor(out=vld[sp, :], in0=vld[sp, :],
                                            in1=sel_pr[sp, CPT - 1:CPT], op=OP.mult)
                    nc.vector.tensor_scalar(out=sel_sl[sp, :], in0=sel_sl[sp, :],
                                            scalar1=float(8 * lc - BIG), scalar2=None, op0=OP.add)
                    nc.vector.tensor_tensor(out=sel_sl[sp, :], in0=sel_sl[sp, :],
                                            in1=vld[sp, :], op=OP.mult)
                    nc.vector.tensor_scalar(out=sel_sl[sp, :], in0=sel_sl[sp, :],
                                            scalar1=BIG, scalar2=None, op0=OP.add)
                    # full-column copies + scatter: other partitions hold the
                    # previous chunks' (slot, token) pairs -> idempotent rewrites
                    nc.vector.tensor_copy(out=sel_sli[:], in_=sel_sl[:])
                    nc.vector.tensor_copy(out=sel_tk[:], in_=sel_ts[:, CPT - 1:CPT])
                    nc.gpsimd.indirect_dma_start(
                        out=slots_scr[:],
                        out_offset=bass.IndirectOffsetOnAxis(ap=sel_sli[:], axis=0),
                        in_=sel_tk[:], in_offset=None,
                        bounds_check=bc_slot, oob_is_err=False,
                    )

        # ---------------- phase B: refinement + scans ----------------
        with tc.tile_pool(name="small", bufs=1) as small, \
             tc.tile_pool(name="gref", bufs=1) as gref, \
             tc.tile_pool(name="psS", bufs=2, space="PSUM") as psS, \
             tc.tile_pool(name="psR", bufs=1, space="PSUM") as psR:

            def cross_part_excl(col_pP1, init, op, tagp):
                """[P,1] per-partition values -> exclusive scan over partitions."""
                ps_r = psS.tile([P, 512], F32, tag="sc")
                nc.tensor.matmul(ps_r[0:1, 0:P], lhsT=col_pP1, rhs=ident[:],
                                 start=True, stop=True)
                rowT = small.tile([1, P], F32, tag=tagp + "_rowT")
                nc.vector.tensor_copy(out=rowT[:], in_=ps_r[0:1, 0:P])
                sc = small.tile([1, P], F32, tag=tagp + "_sc")
                nc.vector.tensor_tensor_scan(
                    out=sc[:], data0=rowT[:], data1=zrow128[:],
                    initial=init, op0=op, op1=OP.add,
                )
                exc = small.tile([1, P], F32, tag=tagp + "_exc")
                nc.vector.memset(exc[0:1, 0:1], init)
                nc.vector.tensor_copy(out=exc[0:1, 1:P], in_=sc[0:1, 0:P - 1])
                ps_b = psS.tile([P, 512], F32, tag="sc")
                nc.tensor.matmul(ps_b[:, 0:1], lhsT=exc[:], rhs=ones_1x1[:],
                                 start=True, stop=True)
                offc = small.tile([P, 1], F32, tag=tagp + "_off")
                nc.vector.tensor_copy(out=offc[:], in_=ps_b[:, 0:1])
                return offc

            # --- refine: exact fp32 logits for the gathered tokens ---
            nc.sync.dma_start(out=slots_sb[:], in_=slots_scr[:])
            nc.gpsimd.indirect_dma_start(
                out=xg[:], out_offset=None, in_=x_d[:],
                in_offset=bass.IndirectOffsetOnAxis(ap=slots_sb[:], axis=0),
                bounds_check=bc_rows, oob_is_err=False,
            )
            nc.gpsimd.indirect_dma_start(
                out=lgst_g[:], out_offset=None, in_=lgst_scr[:],
                in_offset=bass.IndirectOffsetOnAxis(ap=slots_sb[:], axis=0),
                bounds_check=bc_rows, oob_is_err=False,
            )
            xgT = gref.tile([P, ND, K_REF], F32, tag="xgT")
            for j in range(ND):
                ps_t = psS.tile([P, 512], F32, tag="sc")
                nc.tensor.transpose(
                    out=ps_t[:, 0:K_REF],
                    in_=xg[0:K_REF, j * P:(j + 1) * P],
                    identity=ident[0:K_REF, 0:K_REF],
                )
                nc.vector.tensor_copy(out=xgT[:, j, :], in_=ps_t[:, 0:K_REF])
            # h[token, o] = relu(sum_i x[t,i] W1[i,o] + b1[o]) via wide matmuls
            psr = psR.tile([K_REF, D], F32, tag="hR")
            for i_ in range(ND):
                for h in range(2):
                    nc.tensor.matmul(
                        psr[:, h * 512:(h + 1) * 512],
                        lhsT=xgT[:, i_, :],
                        rhs=w1_sb[:, i_, h * 512:(h + 1) * 512],
                        start=(i_ == 0), stop=(i_ == ND - 1),
                    )
            hRb = gref.tile([K_REF, D], F32, tag="hRb")
            nc.vector.tensor_tensor(out=hRb[:], in0=psr[:], in1=b1_bc[:], op=OP.add)
            hR = gref.tile([K_REF, D], F32, tag="hR")
            nc.scalar.activation(out=hR[:], in_=hRb[:], func=ACT.Relu,
                                 bias=zero_col[0:K_REF, :], scale=1.0)
            hw = gref.tile([K_REF, D], F32, tag="hw")
            nc.vector.tensor_tensor(out=hw[:], in0=hR[:], in1=w2_bc[:], op=OP.mult)
            zscan = gref.tile([K_REF, D], F32, tag="zscan")
            nc.vector.tensor_tensor_scan(
                out=zscan[:], data0=hw[:], data1=zeros_kd[:],
                initial=0.0, op0=OP.add, op1=OP.add,
            )
            zex_col = gref.tile([K_REF, 1], F32, tag="zex_col")
            nc.vector.tensor_tensor(out=zex_col[:], in0=zscan[:, D - 1:D],
                                    in1=b2_bc[:], op=OP.add)
            nc.vector.tensor_tensor(out=zex_col[:], in0=zex_col[:], in1=lgst_g[:], op=OP.add)
            bitf = gref.tile([K_REF, 1], F32, tag="bitf")
            nc.vector.tensor_scalar(out=bitf[:], in0=zex_col[:], scalar1=0.0,
                                    scalar2=None, op0=OP.is_gt)
            nc.gpsimd.indirect_dma_start(
                out=hard_scr[:], out_offset=bass.IndirectOffsetOnAxis(ap=slots_sb[:], axis=0),
                in_=bitf[:], in_offset=None,
                bounds_check=bc_rows, oob_is_err=False,
            )

            # --- scans on corrected bits ---
            hard2 = small.tile([P, CPT], F32, tag="hard2")
            nc.sync.dma_start(out=hard2[:], in_=hard_scr[:, 0].rearrange("(p c) -> p c", c=CPT))

            def cross_part_max_scan(inclusive, tagp):
                offm = cross_part_excl(inclusive[:, CPT - 1:CPT], -1.0, OP.max, tagp)
                out_t = small.tile([P, CPT], F32, tag=tagp + "_out")
                nc.vector.tensor_scalar(
                    out=out_t[:], in0=inclusive[:], scalar1=offm[:], scalar2=None, op0=OP.max,
                )
                return out_t, offm

            mi = small.tile([P, CPT], F32, tag="mi")
            nc.vector.tensor_tensor(out=mi[:], in0=iotp1[:], in1=hard2[:], op=OP.mult)
            nc.vector.tensor_scalar(out=mi[:], in0=mi[:], scalar1=-1.0, scalar2=None, op0=OP.add)
            s1l = small.tile([P, CPT], F32, tag="s1l")
            nc.vector.tensor_tensor_scan(
                out=s1l[:], data0=mi[:], data1=zeros_cpt[:],
                initial=-1.0, op0=OP.max, op1=OP.add,
            )
            lb_inc, offm1 = cross_part_max_scan(s1l, "s1")

            lbm1 = small.tile([P, CPT], F32, tag="lbm1")
            nc.vector.tensor_copy(out=lbm1[:, 0:1], in_=offm1[:])
            nc.vector.tensor_copy(out=lbm1[:, 1:CPT], in_=lb_inc[:, 0:CPT - 1])
            mi2 = small.tile([P, CPT], F32, tag="mi2")
            nc.vector.tensor_scalar(out=mi2[:], in0=lbm1[:], scalar1=1.0, scalar2=None, op0=OP.add)
            nc.vector.tensor_tensor(out=mi2[:], in0=mi2[:], in1=hard2[:], op=OP.mult)
            nc.vector.tensor_scalar(out=mi2[:], in0=mi2[:], scalar1=-1.0, scalar2=None, op0=OP.add)
            s2l = small.tile([P, CPT], F32, tag="s2l")
            nc.vector.tensor_tensor_scan(
                out=s2l[:], data0=mi2[:], data1=zeros_cpt[:],
                initial=-1.0, op0=OP.max, op1=OP.add,
            )
            pb, _ = cross_part_max_scan(s2l, "s2")

            cnt = small.tile([P, CPT], F32, tag="cnt")
            nc.vector.tensor_tensor(out=cnt[:], in0=lb_inc[:], in1=pb[:], op=OP.subtract)
            nc.vector.tensor_scalar(out=cnt[:], in0=cnt[:], scalar1=1e-9, scalar2=None, op0=OP.add)
            r_tok = small.tile([P, CPT], F32, tag="r_tok")
            nc.vector.reciprocal(out=r_tok[:], in_=cnt[:])
            # force r = 1 for null tokens (e = -1) so it stays finite in fp16
            mask0 = small.tile([P, CPT], F32, tag="mask0")
            nc.vector.tensor_scalar(out=mask0[:], in0=lb_inc[:], scalar1=-0.5,
                                    scalar2=None, op0=OP.is_gt)
            nc.vector.tensor_scalar(out=r_tok[:], in0=r_tok[:], scalar1=-1.0, scalar2=None, op0=OP.add)
            nc.vector.tensor_tensor(out=r_tok[:], in0=r_tok[:], in1=mask0[:], op=OP.mult)
            nc.vector.tensor_scalar(out=r_tok[:], in0=r_tok[:], scalar1=1.0, scalar2=None, op0=OP.add)

            nc.sync.dma_start(out=e_scr[:, 0].rearrange("(p c) -> p c", c=CPT), in_=lb_inc[:])
            nc.sync.dma_start(out=s_scr[:, 0].rearrange("(p c) -> p c", c=CPT), in_=pb[:])
            nc.sync.dma_start(out=r_scr[:, 0].rearrange("(p c) -> p c", c=CPT), in_=r_tok[:])

        # ---------------- phase C: banded matmul up = diag(r) (M01 @ x) ----------------
        with tc.tile_pool(name="esr", bufs=1) as esr, \
             tc.tile_pool(name="mtp", bufs=8) as mtp, \
             tc.tile_pool(name="upp", bufs=3) as upp, \
             tc.tile_pool(name="psB", bufs=2, space="PSUM") as psB, \
             tc.tile_pool(name="psU", bufs=2, space="PSUM") as psU:
            # full rows of e/s (fp32 -> fp16 for the PE broadcast), r and e as
            # token-major columns for the final scale / null blend
            e_full = esr.tile([1, L], F32, tag="e_full")
            nc.sync.dma_start(out=e_full[:], in_=e_scr[:, 0].rearrange("(a l) -> a l", a=1))
            s_full = esr.tile([1, L], F32, tag="s_full")
            nc.sync.dma_start(out=s_full[:], in_=s_scr[:, 0].rearrange("(a l) -> a l", a=1))
            e_fh = esr.tile([1, L], F16, tag="e_fh")
            nc.vector.tensor_copy(out=e_fh[:], in_=e_full[:])
            s_fh = esr.tile([1, L], F16, tag="s_fh")
            nc.vector.tensor_copy(out=s_fh[:], in_=s_full[:])
            r_col = esr.tile([P, NLT], F32, tag="r_col")
            nc.sync.dma_start(out=r_col[:], in_=r_scr[:, 0].rearrange("(t p) -> p t", p=P))
            e_col0 = esr.tile([P, 1], F32, tag="e_col0")
            nc.sync.dma_start(out=e_col0[:], in_=e_scr[0:P, 0].rearrange("(a p) -> p a", a=1))
            m0col = esr.tile([P, 1], F32, tag="m0")
            nc.vector.tensor_scalar(out=m0col[:], in0=e_col0[:], scalar1=-0.5,
                                    scalar2=None, op0=OP.is_gt)

            for lc in range(NLC):
                lsl = slice(lc * LCH, (lc + 1) * LCH)
                psb2 = psB.tile([P, 2 * LCH], F32, tag="bc")
                nc.tensor.matmul(psb2[:, 0:LCH], lhsT=ones_row_h[:], rhs=e_fh[0:1, lsl],
                                 start=True, stop=True)
                nc.tensor.matmul(psb2[:, LCH:2 * LCH], lhsT=ones_row_h[:], rhs=s_fh[0:1, lsl],
                                 start=True, stop=True)

                # build all masks for the chunk first (DVE), then run the
                # matmuls back-to-back (keeps the PE stream dense)
                mts = {}
                for tli in range(LCH // P):
                    tl = lc * (LCH // P) + tli
                    cs = slice(tli * P, (tli + 1) * P)
                    cs2 = slice(LCH + tli * P, LCH + (tli + 1) * P)
                    for tj in (tl - 1, tl):
                        if tj < 0:
                            continue
                        cmpS = mtp.tile([P, P], F32, tag="cmpS")
                        nc.vector.tensor_scalar(
                            out=cmpS[:], in0=psb2[:, cs2],
                            scalar1=jv16[:, tj:tj + 1], scalar2=None, op0=OP.is_ge,
                        )
                        mt = mtp.tile([P, P], F16, tag="mt")
                        nc.vector.scalar_tensor_tensor(
                            out=mt[:], in0=psb2[:, cs], scalar=jv16[:, tj:tj + 1],
                            in1=cmpS[:], op0=OP.is_ge, op1=OP.subtract,
                        )
                        mts[(tl, tj)] = mt

                for tli in range(LCH // P):
                    tl = lc * (LCH // P) + tli
                    tjs = [t for t in (tl - 1, tl) if t >= 0]
                    psup = psU.tile([P, D], F32, tag="up")
                    for k, tj in enumerate(tjs):
                        for h in range(2):
                            nc.tensor.matmul(
                                psup[:, h * 512:(h + 1) * 512],
                                lhsT=mts[(tl, tj)][:],
                                rhs=x_h[:, tj, h * 512:(h + 1) * 512],
                                start=(k == 0), stop=(k == len(tjs) - 1),
                            )
                    upt = upp.tile([P, D], F32, tag="up")
                    if tl == 0:
                        # upt = (psup*r - null)*m0 + null
                        nc.vector.tensor_scalar(out=upt[:], in0=psup[:],
                                                scalar1=r_col[:, 0:1], scalar2=None, op0=OP.mult)
                        nc.vector.tensor_tensor(out=upt[:], in0=upt[:], in1=null_bc[:], op=OP.subtract)
                        nc.vector.scalar_tensor_tensor(
                            out=upt[:], in0=upt[:], scalar=m0col[:],
                            in1=null_bc[:], op0=OP.mult, op1=OP.add,
                        )
                    else:
                        nc.scalar.activation(out=upt[:], in_=psup[:], func=ACT.Identity,
                                             bias=zero_col[:], scale=r_col[:, tl:tl + 1])
                    nc.sync.dma_start(out=up_d[tl * P:(tl + 1) * P, :], in_=upt[:])

    nc.compile()
    return nc


_nc_cache = {}


def _get_nc(L, D):
    key = (L, D)
    if key not in _nc_cache:
        _nc_cache[key] = build(L, D)
    return _nc_cache[key]


def make_in_maps(inputs, n_cores=N_CORES):
    x = np.ascontiguousarray(np.asarray(inputs["x"], dtype=np.float32))
    noise = np.ascontiguousarray(np.asarray(inputs["noise"], dtype=np.float32))
    shared = {
        "W1": np.ascontiguousarray(np.asarray(inputs["W1"], dtype=np.float32)),
        "b1": np.ascontiguousarray(np.asarray(inputs["b1"], dtype=np.float32)),
        "W2": np.ascontiguousarray(np.asarray(inputs["W2"], dtype=np.float32)),
        "b2": np.ascontiguousarray(np.asarray(inputs["b2"], dtype=np.float32)),
        "null_group": np.ascontiguousarray(np.asarray(inputs["null_group"], dtype=np.float32)),
    }
    return [dict(shared, x=x[c], noise=noise[c]) for c in range(n_cores)]


def kernel(**inputs):
    from concourse.bass_utils import run_bass_kernel_spmd

    x = np.asarray(inputs["x"])
    b, L, D = x.shape
    assert b == N_CORES
    nc = _get_nc(L, D)
    in_maps = make_in_maps(inputs)
    res = run_bass_kernel_spmd(nc, in_maps, core_ids=list(range(N_CORES)))
    out = np.stack([res.results[c]["up"] for c in range(N_CORES)], axis=0)
    return out.astype(np.float32)
